# revision 13
# baseline (speedup 1.0000x reference)
"""OFT block-diagonal rotation forward (nn_Linear_12635793785535).

y = x @ blockdiag(rot_0..rot_63), rot_r = I + 2Q_r + 2Q_r^2 + 2Q_r^3 + 2Q_r^4
with Q_r the skew-symmetric matrix built from weight[r].

Sharding: data-parallel over tokens across 8 NeuronCores; the small derived
rotation blocks are replicated (per the problem's sharding hint).

v2: bf16 datapath on the PE (error budget 2e-2 >> bf16 rounding).
Per core (1024 tokens x 4096 features):
  x arrives f32 in two 1MB slabs per 128-token tile on separate DMA queues
  (SP hardware DGE + Pool software DGE); ACT converts slab0 -> bf16, DVE
  converts slab1. PE transposes bf16 128x128 chunks (1 pass instead of
  f32's 2) into PSUM, DVE copies them packed to SBUF, PE matmuls against
  the replicated bf16 rotation pair-tiles (f32 PSUM accumulate), ACT
  copies y f32 to SBUF, and y drains on the ACT/DVE DMA queues while
  later groups compute. The rotation tiles are DMA'd packed (512KB, only
  the nonzero 64x64 diagonal blocks) into a zeroed [128,32,128] SBUF tile.
Bottleneck target: the ~415 GB/s per-core DMA fabric (16 engines); PE
work is halved vs the f32 baseline so it never gates buffer recycling.
"""

import numpy as np

TOKENS = 8192
FEAT = 4096
R = 64
BLOCK = 64
NPAIR = 32  # pairs of 64-blocks -> 128-wide block-diagonal tiles
GROUP = 4  # pairs per PSUM bank group (4 x 128 = 512 wide)
NGROUP = NPAIR // GROUP  # 8
NUM_TERMS = 5
N_CORES = 8
TOK_SHARD = TOKENS // N_CORES  # 1024
TOK_TILE = 128
N_TTILES = TOK_SHARD // TOK_TILE  # 8
SLAB = 2048  # feature columns per x/y DMA slab
NSLAB = FEAT // SLAB  # 2

_CACHE = {}

# test.py can flip these before calling kernel()
TRACE = False
LAST_RESULTS = None


def _build_bass():
    from contextlib import ExitStack

    import concourse.tile as tile
    from concourse import bacc, mybir
    from concourse.masks import make_identity

    nc = bacc.Bacc(
        "TRN2",
        target_bir_lowering=False,
        debug=False,
        enable_asserts=False,
        num_devices=N_CORES,
    )
    x_d = nc.dram_tensor(
        "x", [TOK_SHARD, FEAT], mybir.dt.float32, kind="ExternalInput"
    ).ap()
    # dense bf16 pair-tiles [k=128, pair, c=128]: contiguous per-partition
    # rows so the DMA is 128 clean 8KB lines (a strided "packed" layout
    # generates 4096 tiny descriptors and stalls the issuing sequencer)
    rot_d = nc.dram_tensor(
        "rot", [128, NPAIR, 128], mybir.dt.bfloat16, kind="ExternalInput"
    ).ap()
    y_d = nc.dram_tensor(
        "y", [TOK_SHARD, FEAT], mybir.dt.float32, kind="ExternalOutput"
    ).ap()

    with tile.TileContext(nc) as tc, ExitStack() as ctx:
        const_pool = ctx.enter_context(tc.tile_pool(name="const", bufs=1))
        xpool = ctx.enter_context(tc.tile_pool(name="xin", bufs=1))
        xbpool = ctx.enter_context(tc.tile_pool(name="xbf", bufs=1))
        ypool = ctx.enter_context(tc.tile_pool(name="yout", bufs=1))
        xtpool = ctx.enter_context(tc.tile_pool(name="xt", bufs=1))
        ps_t = ctx.enter_context(tc.tile_pool(name="ps_t", bufs=4, space="PSUM"))
        ps_y = ctx.enter_context(tc.tile_pool(name="ps_y", bufs=4, space="PSUM"))

        ident = const_pool.tile([128, 128], mybir.dt.bfloat16)
        make_identity(nc, ident)

        # rot rides the ACT queue (idle until the first y drain) so it lands
        # early without delaying the x stream on the SP/Pool queues
        rot_sb = const_pool.tile([128, NPAIR, 128], mybir.dt.bfloat16)
        nc.scalar.dma_start(rot_sb[:], rot_d)

        # x DMAs are issued TWO tiles ahead and converts ONE tile ahead:
        # a convert at the head of the ACT/DVE FIFO queue must never wait on
        # its slab (that would block the tile-t y/xT copies queued behind it)
        def issue_dma(t):
            tok = slice(t * TOK_TILE, (t + 1) * TOK_TILE)
            xs0 = xpool.tile(
                [TOK_TILE, SLAB], mybir.dt.float32, name="xs0", tag="xs0", bufs=5
            )
            nc.sync.dma_start(xs0[:], x_d[tok, 0:SLAB])
            xs1 = xpool.tile(
                [TOK_TILE, SLAB], mybir.dt.float32, name="xs1", tag="xs1", bufs=5
            )
            nc.gpsimd.dma_start(xs1[:], x_d[tok, SLAB : 2 * SLAB])
            return xs0, xs1

        def issue_conv(xs):
            xs0, xs1 = xs
            xb = xbpool.tile(
                [TOK_TILE, FEAT], mybir.dt.bfloat16, name="xb", tag="xb", bufs=3
            )
            nc.scalar.copy(xb[:, 0:SLAB], xs0[:])
            nc.vector.tensor_copy(xb[:, SLAB : 2 * SLAB], xs1[:])
            return xb

        PREFETCH = 4
        xs_tiles = [issue_dma(t) for t in range(PREFETCH)]
        xb_cur = issue_conv(xs_tiles[0])
        xb_next = None

        for t in range(N_TTILES):
            tok = slice(t * TOK_TILE, (t + 1) * TOK_TILE)
            if t + PREFETCH < N_TTILES:
                xs_tiles.append(issue_dma(t + PREFETCH))
            y_slabs = [
                ypool.tile(
                    [TOK_TILE, SLAB], mybir.dt.float32, name=f"ys{s}", tag=f"ys{s}", bufs=3
                )
                for s in range(NSLAB)
            ]
            for g in range(NGROUP):
                s = g // GROUP  # slab index; 4 groups per slab
                gc = (g % GROUP) * GROUP * 128  # column offset within slab
                xt_ps = ps_t.tile([128, GROUP * TOK_TILE], mybir.dt.bfloat16)
                for j in range(GROUP):
                    src = xb_cur[:, g * 512 + j * 128 : g * 512 + (j + 1) * 128]
                    nc.tensor.transpose(
                        xt_ps[:, j * TOK_TILE : (j + 1) * TOK_TILE], src, ident[:]
                    )
                xt_sb = xtpool.tile(
                    [128, GROUP * TOK_TILE], mybir.dt.bfloat16, name="xts", tag="xts", bufs=6
                )
                nc.vector.tensor_copy(xt_sb[:], xt_ps[:])
                y_ps = ps_y.tile([TOK_TILE, GROUP * 128], mybir.dt.float32)
                for j in range(GROUP):
                    p = g * GROUP + j
                    nc.tensor.matmul(
                        y_ps[:, j * 128 : (j + 1) * 128],
                        xt_sb[:, j * TOK_TILE : (j + 1) * TOK_TILE],
                        rot_sb[:, p, :],
                        start=True,
                        stop=True,
                    )
                nc.scalar.copy(y_slabs[s][:, gc : gc + GROUP * 128], y_ps[:])
                if g == GROUP - 1:
                    nc.scalar.dma_start(y_d[tok, 0:SLAB], y_slabs[0][:])
                    # converts for t+1 go mid-tile: their slabs landed tiles
                    # ago, and nothing latency-critical queues behind them
                    if t + 1 < N_TTILES:
                        xb_next = issue_conv(xs_tiles[t + 1])
                elif g == NGROUP - 1:
                    # alternate the second y slab between the SP and Pool
                    # queues so no single queue carries more than ~12.5MB
                    eng = nc.sync if t % 2 == 0 else nc.gpsimd
                    eng.dma_start(y_d[tok, SLAB : 2 * SLAB], y_slabs[1][:])
            xb_cur = xb_next

    nc.compile()
    return nc


def _host_rot_packed(weight):
    """Cayley-Neumann series on host (f32), packed to the nonzero 64x64
    diagonal blocks as bf16 [128, NPAIR, 64] (replicated across cores)."""
    import ml_dtypes

    w = np.asarray(weight, dtype=np.float32)
    rows, cols = np.triu_indices(BLOCK, k=1)
    Q = np.zeros((R, BLOCK, BLOCK), dtype=np.float32)
    Q[:, rows, cols] = w
    Q = Q - np.swapaxes(Q, 1, 2)
    eye = np.eye(BLOCK, dtype=np.float32)
    rot = eye[None, :, :] + 2.0 * Q
    Qp = Q
    for _ in range(2, NUM_TERMS):
        Qp = np.einsum("rij,rjk->rik", Qp, Q).astype(np.float32)
        rot = rot + 2.0 * Qp
    layout = np.zeros((128, NPAIR, 128), dtype=np.float32)
    for pair in range(NPAIR):
        layout[0:64, pair, 0:64] = rot[2 * pair]
        layout[64:128, pair, 64:128] = rot[2 * pair + 1]
    return layout.astype(ml_dtypes.bfloat16)


def kernel(x, weight):
    global LAST_RESULTS
    if "nc" not in _CACHE:
        _CACHE["nc"] = _build_bass()
    nc = _CACHE["nc"]

    from concourse.bass_utils import run_bass_kernel_spmd

    x = np.ascontiguousarray(np.asarray(x, dtype=np.float32))
    rot = _host_rot_packed(weight)
    in_maps = [
        {
            "x": np.ascontiguousarray(x[i * TOK_SHARD : (i + 1) * TOK_SHARD]),
            "rot": rot,
        }
        for i in range(N_CORES)
    ]
    res = run_bass_kernel_spmd(
        nc, in_maps, core_ids=list(range(N_CORES)), trace=TRACE
    )
    LAST_RESULTS = res
    out = np.concatenate([r["y"] for r in res.results], axis=0)
    return out
